# revision 1
# baseline (speedup 1.0000x reference)
"""DiagonalLinear (Toeplitz linear) Trainium2 kernel.

y[b,s,o] = sum_i x[b,s,i] * W[o,i] + bias[o],  W[o,i] = vals[(i-o) mod 8191]
x: [4, 2048, 4096] f32 -> bf16 operands, f32 PSUM/bias/output.

Data-parallel over 8 cores (1024 rows each); per core an 8x8x32 tile loop of
128x512x128 matmuls at the N=512 PE issue floor (215.8ns). The Toeplitz
weight is never materialized: a [128 x 8064] periodic table rv gives every
[128k x 512n] weight tile as a free-dim slice.

v5 vs v2: startup rv loads split across BOTH HWDGE queues, mc1 prefetch
WAW-gated behind the rv tail, ~28 warm-up matmuls on scratch so HAM is at
K=8/8 before the first real MM, and the last tile's drain+store is split
across DVE/GpSimd + both store queues to shorten the tail.
"""

import numpy as np
import ml_dtypes

import bass_rust
import concourse.bass as bass
import concourse.mybir as mybir
import concourse.tile as tile
from concourse.bass_utils import run_bass_kernel_spmd

IN_F = 4096
OUT_F = 4096
NVALS = OUT_F + IN_F - 1  # 8191
B, S = 4, 2048
ROWS = B * S              # 8192
N_CORES = 8
M_PER_CORE = ROWS // N_CORES  # 1024

MT = 128
NT = 512
KT = 128
N_MC = M_PER_CORE // MT   # 8
N_NC = OUT_F // NT        # 8
N_KC = IN_F // KT         # 32
RV_F = (N_KC - 1) * KT + (N_NC - 1) * NT + NT  # 8064
RV_C0 = (N_KC - 1) * KT   # 3968

N_WARM = 36               # dummy matmuls to lift HAM to K=8/8 during DMA wait
                          # (36 x ~107ns cold > 3.4us HAM window, so the gate
                          # flips before the first real matmul issues)

BF16 = mybir.dt.bfloat16

_COMPILED = None


def _legalize_single_wait(nc):
    """This walrus build encodes at most one sync-wait per instruction;
    move extra waits onto carrier NoOps on the same engine."""
    for f in nc.m.functions:
        for blk in f.blocks:
            insts = blk.instructions
            new = []
            changed = False
            for inst in insts:
                si = inst.sync_info
                if si is not None and si.on_wait is not None and len(si.on_wait) > 1:
                    waits = list(si.on_wait)
                    for w in waits[:-1]:
                        nop = mybir.InstNoOp(name=f"I-waitsplit-{nc.next_id()}")
                        nop.engine = inst.engine
                        nop.sync_info = bass_rust.SyncInfo(on_wait=[w], on_update=[])
                        new.append(nop)
                    inst.sync_info = bass_rust.SyncInfo(
                        on_wait=[waits[-1]], on_update=si.on_update
                    )
                    changed = True
                new.append(inst)
            if changed:
                blk.instructions = new


def build_nc():
    f32 = mybir.dt.float32
    nc = bass.Bass()
    # host layout: [mc][p][kc'][m], kc' = N_KC-1-kc (consumption order)
    xt = nc.dram_tensor("xt", [N_MC, 128, N_KC, MT], BF16, kind="ExternalInput")
    rv = nc.dram_tensor("rv", [128, RV_F], BF16, kind="ExternalInput")
    bias_row = nc.dram_tensor("bias_row", [1, OUT_F], f32, kind="ExternalInput")
    y = nc.dram_tensor("y", [M_PER_CORE, OUT_F], f32, kind="ExternalOutput")

    with tile.TileContext(nc) as tc:
        with (
            tc.tile_pool(name="const", bufs=1) as cpool,
            tc.tile_pool(name="xp", bufs=2) as xpool,
            tc.tile_pool(name="op", bufs=4) as opool,
            tc.tile_pool(name="pp", bufs=4, space="PSUM") as ppool,
            tc.tile_pool(name="wm", bufs=1) as wpool,
            tc.tile_pool(name="wp", bufs=1, space="PSUM") as wppool,
        ):
            rv_sb = cpool.tile([128, RV_F], BF16)
            bias_sb = cpool.tile([128, OUT_F], f32)
            xt_first = xpool.tile([128, N_KC, MT], BF16, tag="xt")

            # PE warm-up: short N=128 matmuls on zeroed scratch keep the PE
            # busy from t~7us so the HAM clock gate is at full rate when the
            # first real matmul issues (~3.4us of sustained activity needed).
            warm_sb = wpool.tile([128, 2 * MT], BF16)
            warm_ps = wppool.tile([MT, MT], f32)
            nc.vector.memset(warm_sb, 0)
            for _ in range(N_WARM):
                nc.tensor.matmul(
                    warm_ps, warm_sb[:, 0:MT], warm_sb[:, MT : 2 * MT],
                    start=True, stop=True, skip_group_check=True,
                )

            # Startup loads. First MM needs xt[mc0, kc'=0] + rv[0:512] only.
            # Both HWDGE queues generate descriptors in parallel, finest and
            # soonest-needed chunks first; bulk prefetches are kept off the
            # startup window (mc1 gated below). bias comes in as one 16KB row
            # and is replicated on-chip by gpsimd, not DMA'd as 2MB.
            nc.sync.dma_start(out=xt_first[:, 0:8, :], in_=xt[0, :, 0:8, :])
            nc.scalar.dma_start(out=rv_sb[:, 0:512], in_=rv[:, 0:512])
            nc.sync.dma_start(out=xt_first[:, 8:16, :], in_=xt[0, :, 8:16, :])
            nc.scalar.dma_start(out=rv_sb[:, 512:1024], in_=rv[:, 512:1024])
            nc.sync.dma_start(out=rv_sb[:, 1024:2048], in_=rv[:, 1024:2048])
            nc.scalar.dma_start(out=rv_sb[:, 2048:3072], in_=rv[:, 2048:3072])
            nc.sync.dma_start(out=xt_first[:, 16:32, :], in_=xt[0, :, 16:32, :])
            nc.scalar.dma_start(out=rv_sb[:, 3072:4608], in_=rv[:, 3072:4608])
            nc.sync.dma_start(out=rv_sb[:, 4608:6144], in_=rv[:, 4608:6144])
            # bias: 16KB in DRAM, broadcast across partitions by the DMA AP
            nc.scalar.dma_start(
                out=bias_sb,
                in_=bias_row[0:1, :].partition_broadcast(128).squeeze(1),
            )
            nc.sync.dma_start(out=rv_sb[:, 6144:RV_F], in_=rv[:, 6144:RV_F])

            for mc in range(N_MC):
                m0 = mc * MT
                if mc == 0:
                    xt_sb = xt_first
                else:
                    xt_sb = xpool.tile([128, N_KC, MT], BF16, tag="xt")
                    if mc == 1:
                        # WAW-gate the mc1 prefetch behind the rv tail load so
                        # its 1MB of descriptors can't crowd startup-critical
                        # loads on the shared DMA engines (needed at ~+55us).
                        nc.vector.tensor_copy(
                            xt_sb[0:1, 0, 0:1], rv_sb[0:1, RV_F - 1 : RV_F]
                        )
                    nc.sync.dma_start(out=xt_sb, in_=xt[mc, :, :, :])
                for ncol in range(N_NC):
                    n0 = ncol * NT
                    acc = ppool.tile([MT, NT], f32, tag="acc")
                    for kk in range(N_KC):
                        c = n0 + KT * kk
                        nc.tensor.matmul(
                            acc,
                            xt_sb[:, kk, :],
                            rv_sb[:, c : c + NT],
                            start=(kk == 0),
                            stop=(kk == N_KC - 1),
                        )
                    last = mc == N_MC - 1 and ncol == N_NC - 1
                    if not last:
                        out_sb = opool.tile([MT, NT], f32, tag="out")
                        nc.vector.tensor_add(out_sb, acc, bias_sb[:, n0 : n0 + NT])
                        nc.scalar.dma_start(
                            out=y[m0 : m0 + MT, n0 : n0 + NT], in_=out_sb
                        )
                    else:
                        # tail: split the final drain across DVE+GpSimd and
                        # both store queues so the post-matmul latency is
                        # half a tile, not a full one.
                        out_sb = opool.tile([MT, NT], f32, tag="out")
                        h = NT // 2
                        nc.vector.tensor_add(
                            out_sb[:, 0:h], acc[:, 0:h], bias_sb[:, n0 : n0 + h]
                        )
                        nc.scalar.dma_start(
                            out=y[m0 : m0 + MT, n0 : n0 + h], in_=out_sb[:, 0:h]
                        )
                        nc.vector.tensor_add(
                            out_sb[:, h:NT], acc[:, h:NT],
                            bias_sb[:, n0 + h : n0 + NT],
                        )
                        nc.sync.dma_start(
                            out=y[m0 : m0 + MT, n0 + h : n0 + NT],
                            in_=out_sb[:, h:NT],
                        )
    _legalize_single_wait(nc)
    return nc


def _prep_shared(diagonals, bias):
    vals = np.concatenate([diagonals[OUT_F - 1 :], diagonals[: OUT_F - 1]])
    vals16 = vals.astype(ml_dtypes.bfloat16)
    p = np.arange(128)[:, None]
    u = np.arange(RV_F)[None, :]
    rv = np.ascontiguousarray(vals16[(RV_C0 + p - u) % NVALS])
    bias_rep = np.ascontiguousarray(
        np.broadcast_to(bias.astype(np.float32), (128, OUT_F))
    )
    return rv, bias_rep


def make_in_maps(x, diagonals, bias):
    x = np.asarray(x, dtype=np.float32)
    diagonals = np.asarray(diagonals, dtype=np.float32)
    bias = np.asarray(bias, dtype=np.float32)

    rv, _ = _prep_shared(diagonals, bias)
    bias_row = np.ascontiguousarray(bias.astype(np.float32).reshape(1, OUT_F))
    x16 = x.reshape(ROWS, IN_F).astype(ml_dtypes.bfloat16)

    in_maps = []
    for c in range(N_CORES):
        xc = x16[c * M_PER_CORE : (c + 1) * M_PER_CORE]  # [1024, 4096]
        x4 = xc.reshape(N_MC, MT, N_KC, KT)      # [mc, m, kc, p]
        x4 = x4[:, :, ::-1, :]                   # kc -> kc' (consumption order)
        x4 = np.ascontiguousarray(x4.transpose(0, 3, 2, 1))  # [mc, p, kc', m]
        in_maps.append({"xt": x4, "rv": rv, "bias_row": bias_row})
    return in_maps


def kernel(x, diagonals, bias):
    global _COMPILED
    if _COMPILED is None:
        _COMPILED = build_nc()
    nc = _COMPILED

    in_maps = make_in_maps(x, diagonals, bias)
    res = run_bass_kernel_spmd(nc, in_maps, core_ids=list(range(N_CORES)))
    y = np.concatenate([res.results[c]["y"] for c in range(N_CORES)], axis=0)
    return y.reshape(B, S, OUT_F)



# revision 3
# speedup vs baseline: 1.2219x; 1.2219x over previous
"""DiagonalLinear (Toeplitz linear) Trainium2 kernel — Karatsuba v6.

y[b,s,o] = sum_i x[b,s,i] * W[o,i] + bias[o],  W[o,i] = vals[(i-o) mod 8191]
x: [4, 2048, 4096] f32 -> bf16 operands, f32 PSUM, f16 output (cast back on host).

Data-parallel over 8 cores (1024 rows each). Within a core, the 4096x4096
Toeplitz matmul is decomposed with 3 levels of Karatsuba on the 2x2 block
structure W = [[A,B],[C,A]] (diagonal blocks of a Toeplitz matrix repeat):
  y_left  = A x0 + B x1 = P0 + P2,   P0 = A(x0+x1), P2 = (B-A) x1
  y_right = C x0 + A x1 = P0 + P3,   P3 = (C-A) x0
Recursing 3 times gives 27 leaf products of [512x512] Toeplitz blocks
= 108 N=512 matmuls per 128-row tile instead of 256 (42% of the MACs).
Each leaf's Toeplitz block is a free-dim slice of a [128 x 896] periodic
table built host-side from +/- combinations of shifted `vals`.

Input combos (x0+x1 tree) run on DVE in bf16; the output recombination
tree runs in f16 split across Scalar (P0 PSUM drains), DVE and GpSimd,
with bias folded into the final adds. Rel err ~5.8e-3 (gate 2e-2).
"""

import numpy as np
import ml_dtypes

import bass_rust
import concourse.bass as bass
import concourse.mybir as mybir
import concourse.tile as tile
from concourse.bass_utils import run_bass_kernel_spmd

IN_F = 4096
OUT_F = 4096
NVALS = OUT_F + IN_F - 1  # 8191
B, S = 4, 2048
ROWS = B * S              # 8192
N_CORES = 8
M_PER_CORE = ROWS // N_CORES  # 1024

MT = 128
N_MC = M_PER_CORE // MT   # 8 row-tiles per core
N_KC = IN_F // 128        # 32 k-chunks of 128
LW = 512                  # Karatsuba leaf width
LKC = LW // 128           # 4 k-chunks per leaf
TBW = (LKC - 1) * 128 + LW  # 896: leaf table width
N_LEAF = 27

N_WARM = 36               # PE warm-up matmuls during startup DMA wait

BF16 = mybir.dt.bfloat16
F16 = mybir.dt.float16
F32 = mybir.dt.float32

# L2-node processing order (s1, s2); slice-only nodes first so the first
# matmuls need only the tail quarter of xt. Leaves within a node: s3 in
# (2, 3, 0) so the two slice leaves issue while the s3 combo add runs.
NODE_ORDER = [(2, 2), (2, 3), (2, 0), (3, 2), (3, 3), (3, 0),
              (0, 2), (0, 3), (0, 0)]
LEAF_A3 = (2, 3, 0)
LEAF_ORDER = [(a1, a2, a3) for (a1, a2) in NODE_ORDER for a3 in LEAF_A3]

_COMPILED = None


def _leaf_gens():
    """Leaf generators as {shift: coeff} over v(t) = vals[t mod 8191]."""
    gens = {}

    def sub(a, b):
        r = dict(a)
        for s, c in b.items():
            r[s] = r.get(s, 0) - c
            if r[s] == 0:
                del r[s]
        return r

    def rec(gen, w, path):
        if w == LW:
            gens[path] = gen
            return
        h = w // 2
        g_b = {s + h: c for s, c in gen.items()}
        g_c = {s - h: c for s, c in gen.items()}
        rec(gen, h, path + (0,))
        rec(sub(g_b, gen), h, path + (2,))
        rec(sub(g_c, gen), h, path + (3,))

    rec({0: 1}, IN_F, ())
    return gens


def _build_tables(diagonals):
    """[27, 128, 896] bf16 leaf tables; tbl[p, u] = g(p - u + 384)."""
    vals = np.concatenate(
        [diagonals[OUT_F - 1:], diagonals[: OUT_F - 1]]
    ).astype(np.float64)
    gens = _leaf_gens()
    t_idx = np.arange(-(LW - 1), LW)
    p = np.arange(128)[:, None]
    u = np.arange(TBW)[None, :]
    tbls = np.zeros((N_LEAF, 128, TBW), np.float64)
    for li, path in enumerate(LEAF_ORDER):
        g = np.zeros(2 * LW - 1)
        for s, c in gens[path].items():
            g += c * vals[np.mod(t_idx + s, NVALS)]
        tbls[li] = g[(p - u + 384) + (LW - 1)]
    return np.ascontiguousarray(tbls.astype(ml_dtypes.bfloat16))


def _legalize_single_wait(nc):
    """This walrus build encodes at most one sync-wait per instruction;
    move extra waits onto carrier NoOps on the same engine."""
    for f in nc.m.functions:
        for blk in f.blocks:
            insts = blk.instructions
            new = []
            changed = False
            for inst in insts:
                si = inst.sync_info
                if si is not None and si.on_wait is not None and len(si.on_wait) > 1:
                    waits = list(si.on_wait)
                    for w in waits[:-1]:
                        nop = mybir.InstNoOp(name=f"I-waitsplit-{nc.next_id()}")
                        nop.engine = inst.engine
                        nop.sync_info = bass_rust.SyncInfo(on_wait=[w], on_update=[])
                        new.append(nop)
                    inst.sync_info = bass_rust.SyncInfo(
                        on_wait=[waits[-1]], on_update=si.on_update
                    )
                    changed = True
                new.append(inst)
            if changed:
                blk.instructions = new


def build_nc():
    nc = bass.Bass()
    # host layout: [mc][p][kc][m], kc ascending
    xt = nc.dram_tensor("xt", [N_MC, 128, N_KC, MT], BF16, kind="ExternalInput")
    tbl = nc.dram_tensor("tbl", [N_LEAF, 128, TBW], BF16, kind="ExternalInput")
    bias_row = nc.dram_tensor("bias_row", [1, OUT_F], F16, kind="ExternalInput")
    y = nc.dram_tensor("y", [M_PER_CORE, OUT_F], F16, kind="ExternalOutput")

    with tile.TileContext(nc) as tc:
        with (
            tc.tile_pool(name="const", bufs=1) as cpool,
            tc.tile_pool(name="xp", bufs=2) as xpool,
            tc.tile_pool(name="cb", bufs=2) as cbpool,
            tc.tile_pool(name="l2", bufs=2) as l2pool,
            tc.tile_pool(name="l1", bufs=2) as l1pool,
            tc.tile_pool(name="ot", bufs=2) as opool,
            tc.tile_pool(name="sp", bufs=3) as spool,
            tc.tile_pool(name="pp", bufs=2, space="PSUM") as ppool,
            tc.tile_pool(name="wm", bufs=1) as wpool,
            tc.tile_pool(name="wp", bufs=1, space="PSUM") as wppool,
        ):
            tbl_sb = cpool.tile([128, N_LEAF, TBW], BF16)
            bias_sb = cpool.tile([128, OUT_F], F16)
            xt_first = xpool.tile([128, N_KC, MT], BF16, tag="xt")

            # PE warm-up on zeroed scratch so the HAM clock gate is at full
            # rate when the first real matmul issues.
            warm_sb = wpool.tile([128, 2 * MT], BF16)
            warm_ps = wppool.tile([MT, MT], F32)
            nc.vector.memset(warm_sb, 0)
            for _ in range(N_WARM):
                nc.tensor.matmul(
                    warm_ps, warm_sb[:, 0:MT], warm_sb[:, MT : 2 * MT],
                    start=True, stop=True, skip_group_check=True,
                )

            # Startup loads, finest/soonest-needed first, split across both
            # HWDGE queues. First leaves need xt kc 24:32 + tables 0..2.
            nc.sync.dma_start(out=xt_first[:, 24:32, :], in_=xt[0, :, 24:32, :])
            nc.scalar.dma_start(out=tbl_sb[:, 0, :], in_=tbl[0])
            nc.sync.dma_start(out=tbl_sb[:, 1, :], in_=tbl[1])
            nc.scalar.dma_start(out=tbl_sb[:, 2, :], in_=tbl[2])
            nc.sync.dma_start(out=xt_first[:, 16:24, :], in_=xt[0, :, 16:24, :])
            nc.scalar.dma_start(out=tbl_sb[:, 3, :], in_=tbl[3])
            nc.sync.dma_start(out=tbl_sb[:, 4, :], in_=tbl[4])
            nc.scalar.dma_start(out=tbl_sb[:, 5, :], in_=tbl[5])
            nc.sync.dma_start(out=xt_first[:, 8:16, :], in_=xt[0, :, 8:16, :])
            nc.scalar.dma_start(
                out=bias_sb,
                in_=bias_row[0:1, :].partition_broadcast(128).squeeze(1),
            )
            nc.sync.dma_start(out=tbl_sb[:, 6, :], in_=tbl[6])
            nc.scalar.dma_start(out=tbl_sb[:, 7, :], in_=tbl[7])
            nc.sync.dma_start(out=xt_first[:, 0:8, :], in_=xt[0, :, 0:8, :])
            nc.scalar.dma_start(out=tbl_sb[:, 8, :], in_=tbl[8])
            for li in range(9, N_LEAF):
                eng = nc.sync if li % 2 == 1 else nc.scalar
                eng.dma_start(out=tbl_sb[:, li, :], in_=tbl[li])

            for mc in range(N_MC):
                m0 = mc * MT
                if mc == 0:
                    xt_sb = xt_first
                else:
                    xt_sb = xt_pref
                if mc + 1 < N_MC:
                    xt_pref = xpool.tile([128, N_KC, MT], BF16, tag="xt")
                    nc.sync.dma_start(out=xt_pref, in_=xt[mc + 1, :, :, :])

                # input combo tree (bf16, DVE)
                s1 = cbpool.tile([128, 16, MT], BF16, tag="s1")
                nc.gpsimd.tensor_add(s1, xt_sb[:, 0:16, :], xt_sb[:, 16:32, :])

                l2outs = {}
                l1outs = {}
                for ni, (a1, a2) in enumerate(NODE_ORDER):
                    u_v = {0: s1, 2: xt_sb[:, 16:32, :], 3: xt_sb[:, 0:16, :]}[a1]
                    if a2 == 0:
                        s2 = cbpool.tile([128, 8, MT], BF16, tag=f"s2_{a1}")
                        nc.gpsimd.tensor_add(s2, u_v[:, 0:8, :], u_v[:, 8:16, :])
                        v_v = s2
                    elif a2 == 2:
                        v_v = u_v[:, 8:16, :]
                    else:
                        v_v = u_v[:, 0:8, :]
                    s3 = cbpool.tile([128, 4, MT], BF16, tag=f"s3_{ni}")
                    nc.gpsimd.tensor_add(s3, v_v[:, 0:4, :], v_v[:, 4:8, :])
                    lhss = {2: v_v[:, 4:8, :], 3: v_v[:, 0:4, :], 0: s3}

                    ps = {}
                    for ci, a3 in enumerate(LEAF_A3):
                        li = ni * 3 + ci
                        acc = ppool.tile([128, LW], F32, tag=f"pp{a3}")
                        lhs = lhss[a3]
                        for kk in range(LKC):
                            c = (LKC - 1 - kk) * 128
                            nc.tensor.matmul(
                                acc, lhs[:, kk, :], tbl_sb[:, li, c : c + LW],
                                start=(kk == 0), stop=(kk == LKC - 1),
                            )
                        ps[a3] = acc

                    p0sb = spool.tile([128, LW], F32, tag="p0sb")
                    nc.scalar.copy(p0sb, ps[0])
                    l2t = l2pool.tile([128, 2 * LW], F16, tag=f"c{a2}")
                    nc.vector.tensor_add(l2t[:, 0:LW], ps[2], p0sb)
                    nc.vector.tensor_add(l2t[:, LW : 2 * LW], ps[3], p0sb)
                    l2outs[a2] = l2t

                    if ni % 3 == 2:  # L1 recombination for group a1
                        l1t = l1pool.tile([128, 4 * LW], F16, tag=f"u{a1}")
                        nc.gpsimd.tensor_add(
                            l1t[:, 0 : 2 * LW], l2outs[0], l2outs[2]
                        )
                        nc.gpsimd.tensor_add(
                            l1t[:, 2 * LW : 4 * LW], l2outs[0], l2outs[3]
                        )
                        l1outs[a1] = l1t

                # root recombination + bias, f16 out
                h = OUT_F // 2
                tl = opool.tile([128, h], F16, tag="tl")
                tr = opool.tile([128, h], F16, tag="tr")
                nc.vector.tensor_add(tl, l1outs[0], l1outs[2])
                nc.vector.tensor_add(tr, l1outs[0], l1outs[3])
                outl = opool.tile([128, h], F16, tag="ol")
                outr = opool.tile([128, h], F16, tag="or")
                nc.vector.tensor_add(outl, tl, bias_sb[:, 0:h])
                nc.vector.tensor_add(outr, tr, bias_sb[:, h:OUT_F])
                nc.scalar.dma_start(out=y[m0 : m0 + MT, 0:h], in_=outl)
                nc.scalar.dma_start(out=y[m0 : m0 + MT, h:OUT_F], in_=outr)

    _legalize_single_wait(nc)
    return nc


def make_in_maps(x, diagonals, bias):
    x = np.asarray(x, dtype=np.float32)
    diagonals = np.asarray(diagonals, dtype=np.float32)
    bias = np.asarray(bias, dtype=np.float32)

    tbls = _build_tables(diagonals.astype(np.float64))
    bias_row = np.ascontiguousarray(bias.astype(np.float16).reshape(1, OUT_F))
    x16 = x.reshape(ROWS, IN_F).astype(ml_dtypes.bfloat16)

    in_maps = []
    for c in range(N_CORES):
        xc = x16[c * M_PER_CORE : (c + 1) * M_PER_CORE]  # [1024, 4096]
        x4 = xc.reshape(N_MC, MT, N_KC, 128)  # [mc, m, kc, p]
        x4 = np.ascontiguousarray(x4.transpose(0, 3, 2, 1))  # [mc, p, kc, m]
        in_maps.append({"xt": x4, "tbl": tbls, "bias_row": bias_row})
    return in_maps


def kernel(x, diagonals, bias):
    global _COMPILED
    if _COMPILED is None:
        _COMPILED = build_nc()
    nc = _COMPILED

    in_maps = make_in_maps(x, diagonals, bias)
    res = run_bass_kernel_spmd(nc, in_maps, core_ids=list(range(N_CORES)))
    y = np.concatenate(
        [np.asarray(res.results[c]["y"]) for c in range(N_CORES)], axis=0
    )
    return y.astype(np.float32).reshape(B, S, OUT_F)


# revision 4
# speedup vs baseline: 2.0700x; 1.6940x over previous
"""DiagonalLinear (Toeplitz linear) Trainium2 kernel — Karatsuba v6.

y[b,s,o] = sum_i x[b,s,i] * W[o,i] + bias[o],  W[o,i] = vals[(i-o) mod 8191]
x: [4, 2048, 4096] f32 -> bf16 operands, f32 PSUM, f16 output (cast back on host).

Data-parallel over 8 cores (1024 rows each). Within a core, the 4096x4096
Toeplitz matmul is decomposed with 3 levels of Karatsuba on the 2x2 block
structure W = [[A,B],[C,A]] (diagonal blocks of a Toeplitz matrix repeat):
  y_left  = A x0 + B x1 = P0 + P2,   P0 = A(x0+x1), P2 = (B-A) x1
  y_right = C x0 + A x1 = P0 + P3,   P3 = (C-A) x0
Recursing 3 times gives 27 leaf products of [512x512] Toeplitz blocks
= 108 N=512 matmuls per 128-row tile instead of 256 (42% of the MACs).
Each leaf's Toeplitz block is a free-dim slice of a [128 x 896] periodic
table built host-side from +/- combinations of shifted `vals`.

Input combos (x0+x1 tree) run on DVE in bf16; the output recombination
tree runs in f16 split across Scalar (P0 PSUM drains), DVE and GpSimd,
with bias folded into the final adds. Rel err ~5.8e-3 (gate 2e-2).
"""

import numpy as np
import ml_dtypes

import bass_rust
import concourse.bass as bass
import concourse.mybir as mybir
import concourse.tile as tile
from concourse.bass_utils import run_bass_kernel_spmd

IN_F = 4096
OUT_F = 4096
NVALS = OUT_F + IN_F - 1  # 8191
B, S = 4, 2048
ROWS = B * S              # 8192
N_CORES = 8
M_PER_CORE = ROWS // N_CORES  # 1024

MT = 128
N_MC = M_PER_CORE // MT   # 8 row-tiles per core
N_KC = IN_F // 128        # 32 k-chunks of 128
LW = 512                  # Karatsuba leaf width
LKC = LW // 128           # 4 k-chunks per leaf
TBW = (LKC - 1) * 128 + LW  # 896: leaf table width
N_LEAF = 27

N_WARM = 36               # PE warm-up matmuls during startup DMA wait

BF16 = mybir.dt.bfloat16
F16 = mybir.dt.float16
F32 = mybir.dt.float32

# L2-node processing order (s1, s2); slice-only nodes first so the first
# matmuls need only the tail quarter of xt. Leaves within a node: a3 in
# (2, 0, 3): P2 first (its drain starts early), P0 second (drained for
# both adds), P3 last (consumed straight from PSUM by the nr add).
NODE_ORDER = [(2, 2), (2, 3), (2, 0), (3, 2), (3, 3), (3, 0),
              (0, 2), (0, 3), (0, 0)]
LEAF_A3 = (2, 0, 3)
LEAF_ORDER = [(a1, a2, a3) for (a1, a2) in NODE_ORDER for a3 in LEAF_A3]

_COMPILED = None


def _leaf_gens():
    """Leaf generators as {shift: coeff} over v(t) = vals[t mod 8191]."""
    gens = {}

    def sub(a, b):
        r = dict(a)
        for s, c in b.items():
            r[s] = r.get(s, 0) - c
            if r[s] == 0:
                del r[s]
        return r

    def rec(gen, w, path):
        if w == LW:
            gens[path] = gen
            return
        h = w // 2
        g_b = {s + h: c for s, c in gen.items()}
        g_c = {s - h: c for s, c in gen.items()}
        rec(gen, h, path + (0,))
        rec(sub(g_b, gen), h, path + (2,))
        rec(sub(g_c, gen), h, path + (3,))

    rec({0: 1}, IN_F, ())
    return gens


def _build_tables(diagonals):
    """[27, 128, 896] bf16 leaf tables; tbl[p, u] = g(p - u + 384)."""
    vals = np.concatenate(
        [diagonals[OUT_F - 1:], diagonals[: OUT_F - 1]]
    ).astype(np.float64)
    gens = _leaf_gens()
    t_idx = np.arange(-(LW - 1), LW)
    p = np.arange(128)[:, None]
    u = np.arange(TBW)[None, :]
    tbls = np.zeros((N_LEAF, 128, TBW), np.float64)
    for li, path in enumerate(LEAF_ORDER):
        g = np.zeros(2 * LW - 1)
        for s, c in gens[path].items():
            g += c * vals[np.mod(t_idx + s, NVALS)]
        tbls[li] = g[(p - u + 384) + (LW - 1)]
    return np.ascontiguousarray(tbls.astype(ml_dtypes.bfloat16))


def _legalize_single_wait(nc):
    """This walrus build encodes at most one sync-wait per instruction;
    move extra waits onto carrier NoOps on the same engine."""
    for f in nc.m.functions:
        for blk in f.blocks:
            insts = blk.instructions
            new = []
            changed = False
            for inst in insts:
                si = inst.sync_info
                if si is not None and si.on_wait is not None and len(si.on_wait) > 1:
                    waits = list(si.on_wait)
                    for w in waits[:-1]:
                        nop = mybir.InstNoOp(name=f"I-waitsplit-{nc.next_id()}")
                        nop.engine = inst.engine
                        nop.sync_info = bass_rust.SyncInfo(on_wait=[w], on_update=[])
                        new.append(nop)
                    inst.sync_info = bass_rust.SyncInfo(
                        on_wait=[waits[-1]], on_update=si.on_update
                    )
                    changed = True
                new.append(inst)
            if changed:
                blk.instructions = new


def build_nc():
    nc = bass.Bass()
    # host layout: [mc][p][kc][m], kc ascending
    xt = nc.dram_tensor("xt", [N_MC, 128, N_KC, MT], BF16, kind="ExternalInput")
    tbl = nc.dram_tensor("tbl", [N_LEAF, 128, TBW], BF16, kind="ExternalInput")
    bias_row = nc.dram_tensor("bias_row", [1, OUT_F], F16, kind="ExternalInput")
    y = nc.dram_tensor("y", [M_PER_CORE, OUT_F], F16, kind="ExternalOutput")

    with tile.TileContext(nc) as tc:
        with (
            tc.tile_pool(name="const", bufs=1) as cpool,
            tc.tile_pool(name="xp", bufs=2) as xpool,
            tc.tile_pool(name="cb", bufs=2) as cbpool,
            tc.tile_pool(name="l2", bufs=2) as l2pool,
            tc.tile_pool(name="l1", bufs=2) as l1pool,
            tc.tile_pool(name="ot", bufs=2) as opool,
            tc.tile_pool(name="sp", bufs=3) as spool,
            tc.tile_pool(name="pp", bufs=2, space="PSUM") as ppool,
            tc.tile_pool(name="wm", bufs=1) as wpool,
            tc.tile_pool(name="wp", bufs=1, space="PSUM") as wppool,
        ):
            tbl_sb = cpool.tile([128, N_LEAF, TBW], BF16)
            bias_sb = cpool.tile([128, OUT_F], F16)
            xt_first = xpool.tile([128, N_KC, MT], BF16, tag="xt")

            # PE warm-up on zeroed scratch so the HAM clock gate is at full
            # rate when the first real matmul issues.
            warm_sb = wpool.tile([128, 2 * MT], BF16)
            warm_ps = wppool.tile([MT, MT], F32)
            nc.vector.memset(warm_sb, 0)
            for _ in range(N_WARM):
                nc.tensor.matmul(
                    warm_ps, warm_sb[:, 0:MT], warm_sb[:, MT : 2 * MT],
                    start=True, stop=True, skip_group_check=True,
                )

            # Startup loads, finest/soonest-needed first, split across both
            # HWDGE queues. First leaves need xt kc 24:32 + tables 0..2.
            nc.sync.dma_start(out=xt_first[:, 24:32, :], in_=xt[0, :, 24:32, :])
            nc.scalar.dma_start(out=tbl_sb[:, 0, :], in_=tbl[0])
            nc.sync.dma_start(out=tbl_sb[:, 1, :], in_=tbl[1])
            nc.scalar.dma_start(out=tbl_sb[:, 2, :], in_=tbl[2])
            nc.sync.dma_start(out=xt_first[:, 16:24, :], in_=xt[0, :, 16:24, :])
            nc.scalar.dma_start(out=tbl_sb[:, 3, :], in_=tbl[3])
            nc.sync.dma_start(out=tbl_sb[:, 4, :], in_=tbl[4])
            nc.scalar.dma_start(out=tbl_sb[:, 5, :], in_=tbl[5])
            nc.sync.dma_start(out=xt_first[:, 8:16, :], in_=xt[0, :, 8:16, :])
            nc.scalar.dma_start(
                out=bias_sb,
                in_=bias_row[0:1, :].partition_broadcast(128).squeeze(1),
            )
            nc.sync.dma_start(out=tbl_sb[:, 6, :], in_=tbl[6])
            nc.scalar.dma_start(out=tbl_sb[:, 7, :], in_=tbl[7])
            nc.sync.dma_start(out=xt_first[:, 0:8, :], in_=xt[0, :, 0:8, :])
            nc.scalar.dma_start(out=tbl_sb[:, 8, :], in_=tbl[8])
            for li in range(9, N_LEAF):
                eng = nc.sync if li % 2 == 1 else nc.scalar
                eng.dma_start(out=tbl_sb[:, li, :], in_=tbl[li])

            for mc in range(N_MC):
                m0 = mc * MT
                if mc == 0:
                    xt_sb = xt_first
                else:
                    xt_sb = xt_pref
                if mc + 1 < N_MC:
                    xt_pref = xpool.tile([128, N_KC, MT], BF16, tag="xt")
                    nc.sync.dma_start(out=xt_pref, in_=xt[mc + 1, :, :, :])

                # input combo tree (bf16). s1 on GpSimd (long latency,
                # consumed ~15us later by the (0,*) nodes); the rest on DVE
                # where 2-byte SBUF operands hit the fast path.
                s1 = cbpool.tile([128, 16, MT], BF16, tag="s1")
                nc.gpsimd.tensor_add(s1, xt_sb[:, 0:16, :], xt_sb[:, 16:32, :])

                l2outs = {}
                l1outs = {}
                for ni, (a1, a2) in enumerate(NODE_ORDER):
                    u_v = {0: s1, 2: xt_sb[:, 16:32, :], 3: xt_sb[:, 0:16, :]}[a1]
                    if a2 == 0:
                        s2 = cbpool.tile([128, 8, MT], BF16, tag=f"s2_{a1}")
                        nc.vector.tensor_add(s2, u_v[:, 0:8, :], u_v[:, 8:16, :])
                        v_v = s2
                    elif a2 == 2:
                        v_v = u_v[:, 8:16, :]
                    else:
                        v_v = u_v[:, 0:8, :]
                    s3 = cbpool.tile([128, 4, MT], BF16, tag=f"s3_{ni}")
                    nc.vector.tensor_add(s3, v_v[:, 0:4, :], v_v[:, 4:8, :])
                    lhss = {2: v_v[:, 4:8, :], 3: v_v[:, 0:4, :], 0: s3}

                    ps = {}
                    for ci, a3 in enumerate(LEAF_A3):
                        li = ni * 3 + ci
                        acc = ppool.tile([128, LW], F32, tag=f"pp{a3}")
                        lhs = lhss[a3]
                        for kk in range(LKC):
                            c = (LKC - 1 - kk) * 128
                            nc.tensor.matmul(
                                acc, lhs[:, kk, :], tbl_sb[:, li, c : c + LW],
                                start=(kk == 0), stop=(kk == LKC - 1),
                            )
                        ps[a3] = acc
                        if a3 == 2:
                            p2sb = spool.tile([128, LW], F16, tag="p2sb")
                            nc.scalar.copy(p2sb, acc)
                        elif a3 == 0:
                            p0sb = spool.tile([128, LW], F16, tag="p0sb")
                            nc.scalar.copy(p0sb, acc)

                    l2t = l2pool.tile([128, 2 * LW], F16, tag=f"c{a2}")
                    # nl: all-f16 SBUF add (DVE fast path); nr: one mixed
                    # PSUM read, saving a third drain on Scalar.
                    nc.vector.tensor_add(l2t[:, 0:LW], p2sb, p0sb)
                    nc.vector.tensor_add(l2t[:, LW : 2 * LW], ps[3], p0sb)
                    l2outs[a2] = l2t

                    if ni % 3 == 2:  # L1 recombination for group a1
                        l1t = l1pool.tile([128, 4 * LW], F16, tag=f"u{a1}")
                        nc.vector.tensor_add(
                            l1t[:, 0 : 2 * LW], l2outs[0], l2outs[2]
                        )
                        nc.vector.tensor_add(
                            l1t[:, 2 * LW : 4 * LW], l2outs[0], l2outs[3]
                        )
                        l1outs[a1] = l1t

                # root recombination + bias, all f16 on DVE
                h = OUT_F // 2
                tl = opool.tile([128, h], F16, tag="tl")
                tr = opool.tile([128, h], F16, tag="tr")
                nc.vector.tensor_add(tl, l1outs[0], l1outs[2])
                nc.vector.tensor_add(tr, l1outs[0], l1outs[3])
                outl = opool.tile([128, h], F16, tag="ol")
                outr = opool.tile([128, h], F16, tag="or")
                nc.vector.tensor_add(outl, tl, bias_sb[:, 0:h])
                nc.vector.tensor_add(outr, tr, bias_sb[:, h:OUT_F])
                nc.scalar.dma_start(out=y[m0 : m0 + MT, 0:h], in_=outl)
                nc.scalar.dma_start(out=y[m0 : m0 + MT, h:OUT_F], in_=outr)

    _legalize_single_wait(nc)
    return nc


def make_in_maps(x, diagonals, bias):
    x = np.asarray(x, dtype=np.float32)
    diagonals = np.asarray(diagonals, dtype=np.float32)
    bias = np.asarray(bias, dtype=np.float32)

    tbls = _build_tables(diagonals.astype(np.float64))
    bias_row = np.ascontiguousarray(bias.astype(np.float16).reshape(1, OUT_F))
    x16 = x.reshape(ROWS, IN_F).astype(ml_dtypes.bfloat16)

    in_maps = []
    for c in range(N_CORES):
        xc = x16[c * M_PER_CORE : (c + 1) * M_PER_CORE]  # [1024, 4096]
        x4 = xc.reshape(N_MC, MT, N_KC, 128)  # [mc, m, kc, p]
        x4 = np.ascontiguousarray(x4.transpose(0, 3, 2, 1))  # [mc, p, kc, m]
        in_maps.append({"xt": x4, "tbl": tbls, "bias_row": bias_row})
    return in_maps


def kernel(x, diagonals, bias):
    global _COMPILED
    if _COMPILED is None:
        _COMPILED = build_nc()
    nc = _COMPILED

    in_maps = make_in_maps(x, diagonals, bias)
    res = run_bass_kernel_spmd(nc, in_maps, core_ids=list(range(N_CORES)))
    y = np.concatenate(
        [np.asarray(res.results[c]["y"]) for c in range(N_CORES)], axis=0
    )
    return y.astype(np.float32).reshape(B, S, OUT_F)


# revision 6
# speedup vs baseline: 2.3807x; 1.1501x over previous
"""DiagonalLinear (Toeplitz linear) Trainium2 kernel — Karatsuba v6.

y[b,s,o] = sum_i x[b,s,i] * W[o,i] + bias[o],  W[o,i] = vals[(i-o) mod 8191]
x: [4, 2048, 4096] f32 -> bf16 operands, f32 PSUM, f16 output (cast back on host).

Data-parallel over 8 cores (1024 rows each). Within a core, the 4096x4096
Toeplitz matmul is decomposed with 3 levels of Karatsuba on the 2x2 block
structure W = [[A,B],[C,A]] (diagonal blocks of a Toeplitz matrix repeat):
  y_left  = A x0 + B x1 = P0 + P2,   P0 = A(x0+x1), P2 = (B-A) x1
  y_right = C x0 + A x1 = P0 + P3,   P3 = (C-A) x0
Recursing 3 times gives 27 leaf products of [512x512] Toeplitz blocks
= 108 N=512 matmuls per 128-row tile instead of 256 (42% of the MACs).
Each leaf's Toeplitz block is a free-dim slice of a [128 x 896] periodic
table built host-side from +/- combinations of shifted `vals`.

Input combos (x0+x1 tree) run on DVE in bf16; the output recombination
tree runs in f16 split across Scalar (P0 PSUM drains), DVE and GpSimd,
with bias folded into the final adds. Rel err ~5.8e-3 (gate 2e-2).
"""

import numpy as np
import ml_dtypes

import bass_rust
import concourse.bass as bass
import concourse.mybir as mybir
import concourse.tile as tile
from concourse.bass_utils import run_bass_kernel_spmd

IN_F = 4096
OUT_F = 4096
NVALS = OUT_F + IN_F - 1  # 8191
B, S = 4, 2048
ROWS = B * S              # 8192
N_CORES = 8
M_PER_CORE = ROWS // N_CORES  # 1024

MT = 128
N_MC = M_PER_CORE // MT   # 8 row-tiles per core
N_KC = IN_F // 128        # 32 k-chunks of 128
LW = 512                  # Karatsuba leaf width
LKC = LW // 128           # 4 k-chunks per leaf
TBW = (LKC - 1) * 128 + LW  # 896: leaf table width
N_LEAF = 27

N_WARM = 36               # PE warm-up matmuls during startup DMA wait

BF16 = mybir.dt.bfloat16
F16 = mybir.dt.float16
F32 = mybir.dt.float32

# L2-node processing order (s1, s2); slice-only nodes first so the first
# matmuls need only the tail quarter of xt. Leaves within a node: a3 in
# (2, 0, 3): P2 first (its drain starts early), P0 second (drained for
# both adds), P3 last (consumed straight from PSUM by the nr add).
NODE_ORDER = [(2, 2), (2, 3), (2, 0), (3, 2), (3, 3), (3, 0),
              (0, 2), (0, 3), (0, 0)]
LEAF_A3 = (2, 0, 3)
LEAF_ORDER = [(a1, a2, a3) for (a1, a2) in NODE_ORDER for a3 in LEAF_A3]

_COMPILED = None


def _leaf_gens():
    """Leaf generators as {shift: coeff} over v(t) = vals[t mod 8191]."""
    gens = {}

    def sub(a, b):
        r = dict(a)
        for s, c in b.items():
            r[s] = r.get(s, 0) - c
            if r[s] == 0:
                del r[s]
        return r

    def rec(gen, w, path):
        if w == LW:
            gens[path] = gen
            return
        h = w // 2
        g_b = {s + h: c for s, c in gen.items()}
        g_c = {s - h: c for s, c in gen.items()}
        rec(gen, h, path + (0,))
        rec(sub(g_b, gen), h, path + (2,))
        rec(sub(g_c, gen), h, path + (3,))

    rec({0: 1}, IN_F, ())
    return gens


def _build_tables(diagonals):
    """[27, 128, 896] bf16 leaf tables; tbl[p, u] = g(p - u + 384)."""
    vals = np.concatenate(
        [diagonals[OUT_F - 1:], diagonals[: OUT_F - 1]]
    ).astype(np.float64)
    gens = _leaf_gens()
    t_idx = np.arange(-(LW - 1), LW)
    p = np.arange(128)[:, None]
    u = np.arange(TBW)[None, :]
    tbls = np.zeros((N_LEAF, 128, TBW), np.float64)
    for li, path in enumerate(LEAF_ORDER):
        g = np.zeros(2 * LW - 1)
        for s, c in gens[path].items():
            g += c * vals[np.mod(t_idx + s, NVALS)]
        tbls[li] = g[(p - u + 384) + (LW - 1)]
    return np.ascontiguousarray(tbls.astype(ml_dtypes.bfloat16))


def _legalize_single_wait(nc):
    """This walrus build encodes at most one sync-wait per instruction;
    move extra waits onto carrier NoOps on the same engine."""
    for f in nc.m.functions:
        for blk in f.blocks:
            insts = blk.instructions
            new = []
            changed = False
            for inst in insts:
                si = inst.sync_info
                if si is not None and si.on_wait is not None and len(si.on_wait) > 1:
                    waits = list(si.on_wait)
                    for w in waits[:-1]:
                        nop = mybir.InstNoOp(name=f"I-waitsplit-{nc.next_id()}")
                        nop.engine = inst.engine
                        nop.sync_info = bass_rust.SyncInfo(on_wait=[w], on_update=[])
                        new.append(nop)
                    inst.sync_info = bass_rust.SyncInfo(
                        on_wait=[waits[-1]], on_update=si.on_update
                    )
                    changed = True
                new.append(inst)
            if changed:
                blk.instructions = new


def build_nc():
    nc = bass.Bass()
    # host layout: [mc][p][kc][m], kc ascending
    xt = nc.dram_tensor("xt", [N_MC, 128, N_KC, MT], BF16, kind="ExternalInput")
    tbl = nc.dram_tensor("tbl", [N_LEAF, 128, TBW], BF16, kind="ExternalInput")
    bias_row = nc.dram_tensor("bias_row", [1, OUT_F], F16, kind="ExternalInput")
    y = nc.dram_tensor("y", [M_PER_CORE, OUT_F], F16, kind="ExternalOutput")

    with tile.TileContext(nc) as tc:
        with (
            tc.tile_pool(name="const", bufs=1) as cpool,
            tc.tile_pool(name="xp", bufs=2) as xpool,
            tc.tile_pool(name="cb", bufs=2) as cbpool,
            tc.tile_pool(name="l2", bufs=2) as l2pool,
            tc.tile_pool(name="l1", bufs=2) as l1pool,
            tc.tile_pool(name="ot", bufs=2) as opool,
            tc.tile_pool(name="sp", bufs=3) as spool,
            tc.tile_pool(name="pp", bufs=2, space="PSUM") as ppool,
            tc.tile_pool(name="wm", bufs=1) as wpool,
            tc.tile_pool(name="wp", bufs=1, space="PSUM") as wppool,
        ):
            tbl_sb = cpool.tile([128, N_LEAF, TBW], BF16)
            bias_sb = cpool.tile([128, OUT_F], F16)
            xt_first = xpool.tile([128, N_KC, MT], BF16, tag="xt")

            # PE warm-up on zeroed scratch so the HAM clock gate is at full
            # rate when the first real matmul issues.
            warm_sb = wpool.tile([128, 2 * MT], BF16)
            warm_ps = wppool.tile([MT, MT], F32)
            nc.vector.memset(warm_sb, 0)
            for _ in range(N_WARM):
                nc.tensor.matmul(
                    warm_ps, warm_sb[:, 0:MT], warm_sb[:, MT : 2 * MT],
                    start=True, stop=True, skip_group_check=True,
                )

            # Startup loads, finest/soonest-needed first, split across both
            # HWDGE queues. First leaves need xt kc 24:32 + tables 0..2.
            nc.sync.dma_start(out=xt_first[:, 24:32, :], in_=xt[0, :, 24:32, :])
            nc.scalar.dma_start(out=tbl_sb[:, 0, :], in_=tbl[0])
            nc.sync.dma_start(out=tbl_sb[:, 1, :], in_=tbl[1])
            nc.scalar.dma_start(out=tbl_sb[:, 2, :], in_=tbl[2])
            nc.sync.dma_start(out=xt_first[:, 16:24, :], in_=xt[0, :, 16:24, :])
            nc.scalar.dma_start(out=tbl_sb[:, 3, :], in_=tbl[3])
            nc.sync.dma_start(out=tbl_sb[:, 4, :], in_=tbl[4])
            nc.scalar.dma_start(out=tbl_sb[:, 5, :], in_=tbl[5])
            nc.sync.dma_start(out=xt_first[:, 8:16, :], in_=xt[0, :, 8:16, :])
            nc.scalar.dma_start(
                out=bias_sb,
                in_=bias_row[0:1, :].partition_broadcast(128).squeeze(1),
            )
            nc.sync.dma_start(out=tbl_sb[:, 6, :], in_=tbl[6])
            nc.scalar.dma_start(out=tbl_sb[:, 7, :], in_=tbl[7])
            nc.sync.dma_start(out=xt_first[:, 0:8, :], in_=xt[0, :, 0:8, :])
            nc.scalar.dma_start(out=tbl_sb[:, 8, :], in_=tbl[8])
            for li in range(9, N_LEAF):
                eng = nc.sync if li % 2 == 1 else nc.scalar
                eng.dma_start(out=tbl_sb[:, li, :], in_=tbl[li])

            for mc in range(N_MC):
                m0 = mc * MT
                if mc == 0:
                    xt_sb = xt_first
                else:
                    xt_sb = xt_pref
                if mc + 1 < N_MC:
                    xt_pref = xpool.tile([128, N_KC, MT], BF16, tag="xt")
                    nc.sync.dma_start(out=xt_pref, in_=xt[mc + 1, :, :, :])

                # input combo tree (bf16, all DVE: GpSimd shares SBUF ports
                # with DVE and running it concurrently stalls DVE ~6x).
                s1 = cbpool.tile([128, 16, MT], BF16, tag="s1")
                nc.vector.tensor_add(s1, xt_sb[:, 0:16, :], xt_sb[:, 16:32, :])

                l2outs = {}
                l1outs = {}
                for ni, (a1, a2) in enumerate(NODE_ORDER):
                    u_v = {0: s1, 2: xt_sb[:, 16:32, :], 3: xt_sb[:, 0:16, :]}[a1]
                    if a2 == 0:
                        s2 = cbpool.tile([128, 8, MT], BF16, tag=f"s2_{a1}")
                        nc.vector.tensor_add(s2, u_v[:, 0:8, :], u_v[:, 8:16, :])
                        v_v = s2
                    elif a2 == 2:
                        v_v = u_v[:, 8:16, :]
                    else:
                        v_v = u_v[:, 0:8, :]
                    s3 = cbpool.tile([128, 4, MT], BF16, tag=f"s3_{ni}")
                    nc.vector.tensor_add(s3, v_v[:, 0:4, :], v_v[:, 4:8, :])
                    lhss = {2: v_v[:, 4:8, :], 3: v_v[:, 0:4, :], 0: s3}

                    sbs = {}
                    for ci, a3 in enumerate(LEAF_A3):
                        li = ni * 3 + ci
                        acc = ppool.tile([128, LW], F32, tag=f"pp{a3}")
                        lhs = lhss[a3]
                        for kk in range(LKC):
                            c = (LKC - 1 - kk) * 128
                            nc.tensor.matmul(
                                acc, lhs[:, kk, :], tbl_sb[:, li, c : c + LW],
                                start=(kk == 0), stop=(kk == LKC - 1),
                            )
                        psb = spool.tile([128, LW], F16, tag=f"p{a3}sb")
                        nc.scalar.copy(psb, acc)
                        sbs[a3] = psb

                    l2t = l2pool.tile([128, 2 * LW], F16, tag=f"c{a2}")
                    nc.vector.tensor_add(l2t[:, 0:LW], sbs[2], sbs[0])
                    nc.vector.tensor_add(l2t[:, LW : 2 * LW], sbs[3], sbs[0])
                    l2outs[a2] = l2t

                    if ni % 3 == 2:  # L1 recombination for group a1
                        l1t = l1pool.tile([128, 4 * LW], F16, tag=f"u{a1}")
                        nc.vector.tensor_add(
                            l1t[:, 0 : 2 * LW], l2outs[0], l2outs[2]
                        )
                        nc.vector.tensor_add(
                            l1t[:, 2 * LW : 4 * LW], l2outs[0], l2outs[3]
                        )
                        l1outs[a1] = l1t

                # root recombination + bias, all f16 on DVE
                h = OUT_F // 2
                tl = opool.tile([128, h], F16, tag="tl")
                tr = opool.tile([128, h], F16, tag="tr")
                nc.vector.tensor_add(tl, l1outs[0], l1outs[2])
                nc.vector.tensor_add(tr, l1outs[0], l1outs[3])
                outl = opool.tile([128, h], F16, tag="ol")
                outr = opool.tile([128, h], F16, tag="or")
                nc.vector.tensor_add(outl, tl, bias_sb[:, 0:h])
                nc.vector.tensor_add(outr, tr, bias_sb[:, h:OUT_F])
                nc.scalar.dma_start(out=y[m0 : m0 + MT, 0:h], in_=outl)
                nc.scalar.dma_start(out=y[m0 : m0 + MT, h:OUT_F], in_=outr)

    _legalize_single_wait(nc)
    return nc


def make_in_maps(x, diagonals, bias):
    x = np.asarray(x, dtype=np.float32)
    diagonals = np.asarray(diagonals, dtype=np.float32)
    bias = np.asarray(bias, dtype=np.float32)

    tbls = _build_tables(diagonals.astype(np.float64))
    bias_row = np.ascontiguousarray(bias.astype(np.float16).reshape(1, OUT_F))
    x16 = x.reshape(ROWS, IN_F).astype(ml_dtypes.bfloat16)

    in_maps = []
    for c in range(N_CORES):
        xc = x16[c * M_PER_CORE : (c + 1) * M_PER_CORE]  # [1024, 4096]
        x4 = xc.reshape(N_MC, MT, N_KC, 128)  # [mc, m, kc, p]
        x4 = np.ascontiguousarray(x4.transpose(0, 3, 2, 1))  # [mc, p, kc, m]
        in_maps.append({"xt": x4, "tbl": tbls, "bias_row": bias_row})
    return in_maps


def kernel(x, diagonals, bias):
    global _COMPILED
    if _COMPILED is None:
        _COMPILED = build_nc()
    nc = _COMPILED

    in_maps = make_in_maps(x, diagonals, bias)
    res = run_bass_kernel_spmd(nc, in_maps, core_ids=list(range(N_CORES)))
    y = np.concatenate(
        [np.asarray(res.results[c]["y"]) for c in range(N_CORES)], axis=0
    )
    return y.astype(np.float32).reshape(B, S, OUT_F)


# revision 10
# speedup vs baseline: 2.4242x; 1.0183x over previous
"""DiagonalLinear (Toeplitz linear) Trainium2 kernel — Karatsuba v11.

y[b,s,o] = sum_i x[b,s,i] * W[o,i] + bias[o],  W[o,i] = vals[(i-o) mod 8191]
x: [4, 2048, 4096] f32 -> bf16 operands, f32 PSUM, f16 output (cast on host).

Data-parallel over 8 cores (1024 rows each). Within a core, the 4096x4096
Toeplitz matmul is decomposed with 3 levels of Karatsuba on the 2x2 block
structure W = [[A,B],[C,A]] (diagonal blocks of a Toeplitz matrix repeat):
  y_left  = A x0 + B x1 = P0 + P2,   P0 = A(x0+x1), P2 = (B-A) x1
  y_right = C x0 + A x1 = P0 + P3,   P3 = (C-A) x0
Recursing 3x gives 27 leaf products of [512x512] Toeplitz blocks = 108
N=512 matmuls per 128-row tile instead of 256 (42% of the MACs). Each
leaf block is a free-dim slice of a [128 x 896] periodic table built
host-side from +/- combinations of shifted `vals`.

Engine split (measured rates): PE runs the 864 matmuls gap-free at the
215.8ns N=512 issue floor; Scalar drains all 27 leaf PSUMs per row-tile
to f16 SBUF (~690ns each); DVE does every add in 16-bit SBUF (2x mode,
~425ns per 512-col add) — input combo tree, nl/nr, L1, root+bias.
GpSimd is unused for compute: it shares SBUF ports with DVE and running
them concurrently stalls DVE ~6x. The next row-tile's combos are emitted
on DVE before the current tile's tail adds so the PE stream never breaks
at a row-tile boundary. Startup orders table/xt DMAs by first-need on
both HWDGE queues (supply-bound at ~330GB/s); the last row-tile runs
group 0 first and finishes group 3 in column halves to shorten the tail.
Rel err ~5.8e-3 (gate 2e-2).
"""

import numpy as np
import ml_dtypes

import bass_rust
import concourse.bass as bass
import concourse.mybir as mybir
import concourse.tile as tile
from concourse.bass_utils import run_bass_kernel_spmd

IN_F = 4096
OUT_F = 4096
NVALS = OUT_F + IN_F - 1  # 8191
B, S = 4, 2048
ROWS = B * S              # 8192
N_CORES = 8
M_PER_CORE = ROWS // N_CORES  # 1024

MT = 128
N_MC = M_PER_CORE // MT   # 8 row-tiles per core
N_KC = IN_F // 128        # 32 k-chunks of 128
LW = 512                  # Karatsuba leaf width
LKC = LW // 128           # 4 k-chunks per leaf
TBW = (LKC - 1) * 128 + LW  # 896: leaf table width
N_LEAF = 27

N_WARM = 28               # PE warm-up matmuls during startup DMA wait

BF16 = mybir.dt.bfloat16
F16 = mybir.dt.float16
F32 = mybir.dt.float32

# L2-node processing order (a1, a2); slice-only nodes first so the first
# matmuls need only the tail quarter of xt. Leaves within a node: a3 in
# (2, 0, 3): P2 first (its drain starts early), P0 second (drained for
# both adds), P3 last.
NODE_ORDER = [(2, 2), (2, 3), (2, 0), (3, 2), (3, 3), (3, 0),
              (0, 2), (0, 3), (0, 0)]
# last row-tile: group 0 first, group 3 finished half-by-half
LAST_ORDER = [(0, 2), (0, 3), (0, 0), (2, 2), (2, 3), (2, 0),
              (3, 2), (3, 0), (3, 3)]
LEAF_A3 = (2, 0, 3)
LEAF_ORDER = [(a1, a2, a3) for (a1, a2) in NODE_ORDER for a3 in LEAF_A3]
LEAF_IDX = {p: i for i, p in enumerate(LEAF_ORDER)}

_COMPILED = None


def _leaf_gens():
    """Leaf generators as {shift: coeff} over v(t) = vals[t mod 8191]."""
    gens = {}

    def sub(a, b):
        r = dict(a)
        for s, c in b.items():
            r[s] = r.get(s, 0) - c
            if r[s] == 0:
                del r[s]
        return r

    def rec(gen, w, path):
        if w == LW:
            gens[path] = gen
            return
        h = w // 2
        g_b = {s + h: c for s, c in gen.items()}
        g_c = {s - h: c for s, c in gen.items()}
        rec(gen, h, path + (0,))
        rec(sub(g_b, gen), h, path + (2,))
        rec(sub(g_c, gen), h, path + (3,))

    rec({0: 1}, IN_F, ())
    return gens


def _build_tables(diagonals):
    """[27, 128, 896] bf16 leaf tables; tbl[p, u] = g(p - u + 384)."""
    vals = np.concatenate(
        [diagonals[OUT_F - 1:], diagonals[: OUT_F - 1]]
    ).astype(np.float64)
    gens = _leaf_gens()
    t_idx = np.arange(-(LW - 1), LW)
    p = np.arange(128)[:, None]
    u = np.arange(TBW)[None, :]
    tbls = np.zeros((N_LEAF, 128, TBW), np.float64)
    for li, path in enumerate(LEAF_ORDER):
        g = np.zeros(2 * LW - 1)
        for s, c in gens[path].items():
            g += c * vals[np.mod(t_idx + s, NVALS)]
        tbls[li] = g[(p - u + 384) + (LW - 1)]
    return np.ascontiguousarray(tbls.astype(ml_dtypes.bfloat16))


def _legalize_single_wait(nc):
    """This walrus build encodes at most one sync-wait per instruction;
    move extra waits onto carrier NoOps on the same engine."""
    for f in nc.m.functions:
        for blk in f.blocks:
            insts = blk.instructions
            new = []
            changed = False
            for inst in insts:
                si = inst.sync_info
                if si is not None and si.on_wait is not None and len(si.on_wait) > 1:
                    waits = list(si.on_wait)
                    for w in waits[:-1]:
                        nop = mybir.InstNoOp(name=f"I-waitsplit-{nc.next_id()}")
                        nop.engine = inst.engine
                        nop.sync_info = bass_rust.SyncInfo(on_wait=[w], on_update=[])
                        new.append(nop)
                    inst.sync_info = bass_rust.SyncInfo(
                        on_wait=[waits[-1]], on_update=si.on_update
                    )
                    changed = True
                new.append(inst)
            if changed:
                blk.instructions = new


def build_nc():
    nc = bass.Bass()
    # host layout: [mc][p][kc][m], kc ascending
    xt = nc.dram_tensor("xt", [N_MC, 128, N_KC, MT], BF16, kind="ExternalInput")
    tbl = nc.dram_tensor("tbl", [N_LEAF, 128, TBW], BF16, kind="ExternalInput")
    bias_row = nc.dram_tensor("bias_row", [1, OUT_F], F16, kind="ExternalInput")
    y = nc.dram_tensor("y", [M_PER_CORE, OUT_F], F16, kind="ExternalOutput")

    with tile.TileContext(nc) as tc:
        with (
            tc.tile_pool(name="const", bufs=1) as cpool,
            tc.tile_pool(name="xp", bufs=3) as xpool,
            tc.tile_pool(name="cb", bufs=2) as cbpool,
            tc.tile_pool(name="l2", bufs=2) as l2pool,
            tc.tile_pool(name="l1", bufs=2) as l1pool,
            tc.tile_pool(name="ot", bufs=2) as opool,
            tc.tile_pool(name="sp", bufs=4) as spool,
            tc.tile_pool(name="pp", bufs=2, space="PSUM") as ppool,
            tc.tile_pool(name="wm", bufs=1) as wpool,
            tc.tile_pool(name="wp", bufs=1, space="PSUM") as wppool,
        ):
            tbl_sb = cpool.tile([128, N_LEAF, TBW], BF16)
            bias_sb = cpool.tile([128, OUT_F], F16)
            xt_first = xpool.tile([128, N_KC, MT], BF16, tag="xt")

            # PE warm-up on zeroed scratch so the HAM clock gate is at full
            # rate when the first real matmul issues.
            warm_sb = wpool.tile([128, 2 * MT], BF16)
            warm_ps = wppool.tile([MT, MT], F32)
            nc.vector.memset(warm_sb, 0)
            for _ in range(N_WARM):
                nc.tensor.matmul(
                    warm_ps, warm_sb[:, 0:MT], warm_sb[:, MT : 2 * MT],
                    start=True, stop=True, skip_group_check=True,
                )

            # Startup loads in first-need order across both HWDGE queues.
            # Supply is ~330GB/s aggregate; tables+xt0 (7.2MB) must land
            # within ~24us, so nothing else rides the queues before them
            # (xt1 queues behind the tables, xt2+ prefetch in the mc loop).
            nc.sync.dma_start(out=xt_first[:, 24:32, :], in_=xt[0, :, 24:32, :])
            nc.scalar.dma_start(out=tbl_sb[:, 0, 384:TBW], in_=tbl[0, :, 384:TBW])
            nc.sync.dma_start(out=tbl_sb[:, 1, :], in_=tbl[1])
            nc.scalar.dma_start(out=tbl_sb[:, 0, 0:384], in_=tbl[0, :, 0:384])
            nc.sync.dma_start(out=tbl_sb[:, 2, :], in_=tbl[2])
            nc.scalar.dma_start(out=tbl_sb[:, 3, :], in_=tbl[3])
            nc.sync.dma_start(out=xt_first[:, 16:24, :], in_=xt[0, :, 16:24, :])
            nc.scalar.dma_start(out=tbl_sb[:, 4, :], in_=tbl[4])
            nc.sync.dma_start(out=tbl_sb[:, 5, :], in_=tbl[5])
            nc.scalar.dma_start(out=tbl_sb[:, 6, :], in_=tbl[6])
            nc.sync.dma_start(out=tbl_sb[:, 7, :], in_=tbl[7])
            nc.scalar.dma_start(out=xt_first[:, 8:16, :], in_=xt[0, :, 8:16, :])
            nc.sync.dma_start(out=tbl_sb[:, 8, :], in_=tbl[8])
            nc.scalar.dma_start(out=tbl_sb[:, 9, :], in_=tbl[9])
            nc.sync.dma_start(out=tbl_sb[:, 10, :], in_=tbl[10])
            nc.scalar.dma_start(out=xt_first[:, 0:8, :], in_=xt[0, :, 0:8, :])
            nc.sync.dma_start(
                out=bias_sb,
                in_=bias_row[0:1, :].partition_broadcast(128).squeeze(1),
            )
            for li in range(11, N_LEAF):
                eng = nc.scalar if li % 2 else nc.sync
                eng.dma_start(out=tbl_sb[:, li, :], in_=tbl[li])

            def emit_combos(xt_sb):
                """Input combo tree for one row-tile (bf16, all DVE), in
                node-need order so mc0 can start on partially-loaded xt."""
                cb = {}
                u2 = xt_sb[:, 16:32, :]
                cb["s3_22"] = cbpool.tile([128, 4, MT], BF16, tag="s3_22", name="s3_22")
                nc.vector.tensor_add(cb["s3_22"], u2[:, 8:12, :], u2[:, 12:16, :])
                cb["s3_23"] = cbpool.tile([128, 4, MT], BF16, tag="s3_23", name="s3_23")
                nc.vector.tensor_add(cb["s3_23"], u2[:, 0:4, :], u2[:, 4:8, :])
                s2_2 = cbpool.tile([128, 8, MT], BF16, tag="s2_2", name="s2_2")
                nc.vector.tensor_add(s2_2, u2[:, 0:8, :], u2[:, 8:16, :])
                cb["s2_2"] = s2_2
                cb["s3_20"] = cbpool.tile([128, 4, MT], BF16, tag="s3_20", name="s3_20")
                nc.vector.tensor_add(cb["s3_20"], s2_2[:, 0:4, :], s2_2[:, 4:8, :])
                u3 = xt_sb[:, 0:16, :]
                cb["s3_32"] = cbpool.tile([128, 4, MT], BF16, tag="s3_32", name="s3_32")
                nc.vector.tensor_add(cb["s3_32"], u3[:, 8:12, :], u3[:, 12:16, :])
                cb["s3_33"] = cbpool.tile([128, 4, MT], BF16, tag="s3_33", name="s3_33")
                nc.vector.tensor_add(cb["s3_33"], u3[:, 0:4, :], u3[:, 4:8, :])
                s2_3 = cbpool.tile([128, 8, MT], BF16, tag="s2_3", name="s2_3")
                nc.vector.tensor_add(s2_3, u3[:, 0:8, :], u3[:, 8:16, :])
                cb["s2_3"] = s2_3
                cb["s3_30"] = cbpool.tile([128, 4, MT], BF16, tag="s3_30", name="s3_30")
                nc.vector.tensor_add(cb["s3_30"], s2_3[:, 0:4, :], s2_3[:, 4:8, :])
                s1 = cbpool.tile([128, 16, MT], BF16, tag="s1", name="s1")
                nc.vector.tensor_add(s1, xt_sb[:, 0:16, :], xt_sb[:, 16:32, :])
                cb["s1"] = s1
                cb["s3_02"] = cbpool.tile([128, 4, MT], BF16, tag="s3_02", name="s3_02")
                nc.vector.tensor_add(cb["s3_02"], s1[:, 8:12, :], s1[:, 12:16, :])
                cb["s3_03"] = cbpool.tile([128, 4, MT], BF16, tag="s3_03", name="s3_03")
                nc.vector.tensor_add(cb["s3_03"], s1[:, 0:4, :], s1[:, 4:8, :])
                s2_0 = cbpool.tile([128, 8, MT], BF16, tag="s2_0", name="s2_0")
                nc.vector.tensor_add(s2_0, s1[:, 0:8, :], s1[:, 8:16, :])
                cb["s2_0"] = s2_0
                cb["s3_00"] = cbpool.tile([128, 4, MT], BF16, tag="s3_00", name="s3_00")
                nc.vector.tensor_add(cb["s3_00"], s2_0[:, 0:4, :], s2_0[:, 4:8, :])
                return cb

            def process_node(a1, a2, xt_sb, combos):
                """Matmuls + drains + nl/nr adds for one L2 node; returns
                the node's f16 [128, 1024] (nl|nr) output tile."""
                if a2 == 0:
                    v_v = combos[f"s2_{a1}"]
                else:
                    u_v = {0: combos["s1"], 2: xt_sb[:, 16:32, :],
                           3: xt_sb[:, 0:16, :]}[a1]
                    v_v = u_v[:, 8:16, :] if a2 == 2 else u_v[:, 0:8, :]
                s3 = combos[f"s3_{a1}{a2}"]
                lhss = {2: v_v[:, 4:8, :], 3: v_v[:, 0:4, :], 0: s3}

                sbs = {}
                for a3 in LEAF_A3:
                    li = LEAF_IDX[(a1, a2, a3)]
                    acc = ppool.tile(
                        [128, LW], F32, tag=f"pp{a3}", name=f"pp{a3}",
                        bufs=3 if a3 == 0 else 2,
                    )
                    lhs = lhss[a3]
                    for kk in range(LKC):
                        c = (LKC - 1 - kk) * 128
                        nc.tensor.matmul(
                            acc, lhs[:, kk, :], tbl_sb[:, li, c : c + LW],
                            start=(kk == 0), stop=(kk == LKC - 1),
                        )
                    psb = spool.tile([128, LW], F16, tag=f"p{a3}sb", name=f"p{a3}sb")
                    nc.scalar.copy(psb, acc)
                    sbs[a3] = psb

                l2t = l2pool.tile([128, 2 * LW], F16, tag=f"c{a2}", name=f"c{a2}")
                nc.vector.tensor_add(l2t[:, 0:LW], sbs[2], sbs[0])
                nc.vector.tensor_add(l2t[:, LW : 2 * LW], sbs[3], sbs[0])
                return l2t

            def emit_l1(a1, l2outs):
                l1t = l1pool.tile([128, 4 * LW], F16, tag=f"u{a1}", name=f"u{a1}")
                nc.vector.tensor_add(l1t[:, 0 : 2 * LW], l2outs[0], l2outs[2])
                nc.vector.tensor_add(l1t[:, 2 * LW : 4 * LW], l2outs[0], l2outs[3])
                return l1t

            xt_tiles = {0: xt_first}
            combos = emit_combos(xt_first)
            h = OUT_F // 2
            q = OUT_F // 4
            for mc in range(N_MC):
                m0 = mc * MT
                xt_sb = xt_tiles.pop(mc)
                last = mc == N_MC - 1
                # prefetch two ahead so mc+1's combos never wait on DMA
                nxts = (1, 2) if mc == 0 else (mc + 2,)
                for nxt in nxts:
                    if nxt < N_MC:
                        xt_pref = xpool.tile(
                            [128, N_KC, MT], BF16, tag="xt", name="xt_pref"
                        )
                        nc.sync.dma_start(out=xt_pref, in_=xt[nxt, :, :, :])
                        xt_tiles[nxt] = xt_pref

                if not last:
                    l2outs = {}
                    l1outs = {}
                    deferred = None
                    for ni, (a1, a2) in enumerate(NODE_ORDER):
                        l2outs[a2] = process_node(a1, a2, xt_sb, combos)
                        if ni % 3 != 2:
                            continue
                        if a1 == 0:
                            # defer group 0's l1 + root: next row-tile's
                            # combos go first on DVE so the PE rolls over
                            # the row-tile boundary without a stall
                            deferred = dict(l2outs)
                        else:
                            l1outs[a1] = emit_l1(a1, l2outs)

                    combos = emit_combos(xt_tiles[mc + 1])
                    l1outs[0] = emit_l1(0, deferred)

                    tl = opool.tile([128, h], F16, tag="tl")
                    tr = opool.tile([128, h], F16, tag="tr")
                    nc.vector.tensor_add(tl, l1outs[0], l1outs[2])
                    nc.vector.tensor_add(tr, l1outs[0], l1outs[3])
                    outl = opool.tile([128, h], F16, tag="ol")
                    outr = opool.tile([128, h], F16, tag="or")
                    nc.vector.tensor_add(outl, tl, bias_sb[:, 0:h])
                    nc.vector.tensor_add(outr, tr, bias_sb[:, h:OUT_F])
                    nc.scalar.dma_start(out=y[m0 : m0 + MT, 0:h], in_=outl)
                    nc.scalar.dma_start(out=y[m0 : m0 + MT, h:OUT_F], in_=outr)
                    continue

                # ---- last row-tile: minimize post-matmul tail ----
                l2outs = {}
                u0 = None
                for a1, a2 in LAST_ORDER[:6]:
                    l2outs[a2] = process_node(a1, a2, xt_sb, combos)
                    if a2 == 0 and a1 == 0:
                        u0 = emit_l1(0, l2outs)
                    elif a2 == 0 and a1 == 2:
                        u2 = emit_l1(2, l2outs)
                        tl = opool.tile([128, h], F16, tag="tl")
                        nc.vector.tensor_add(tl, u0, u2)
                        outl = opool.tile([128, h], F16, tag="ol")
                        nc.vector.tensor_add(outl, tl, bias_sb[:, 0:h])
                        nc.scalar.dma_start(out=y[m0 : m0 + MT, 0:h], in_=outl)
                c2 = process_node(3, 2, xt_sb, combos)
                c0 = process_node(3, 0, xt_sb, combos)
                # right-left 1024 finishes before the last node's matmuls
                ml = l1pool.tile([128, 2 * LW], F16, tag="u3", name="ml3")
                nc.vector.tensor_add(ml, c0, c2)
                trm = opool.tile([128, q], F16, tag="tr")
                nc.vector.tensor_add(trm, u0[:, 0 : 2 * LW], ml)
                outrm = opool.tile([128, q], F16, tag="or")
                nc.vector.tensor_add(outrm, trm, bias_sb[:, h : h + q])
                nc.scalar.dma_start(out=y[m0 : m0 + MT, h : h + q], in_=outrm)
                c3 = process_node(3, 3, xt_sb, combos)
                mr = l1pool.tile([128, 2 * LW], F16, tag="u2", name="mr3")
                nc.vector.tensor_add(mr, c0, c3)
                trr = opool.tile([128, q], F16, tag="tl", name="trr")
                nc.vector.tensor_add(trr, u0[:, 2 * LW : 4 * LW], mr)
                outrr = opool.tile([128, q], F16, tag="ol", name="outrr")
                nc.vector.tensor_add(outrr, trr, bias_sb[:, h + q : OUT_F])
                nc.sync.dma_start(out=y[m0 : m0 + MT, h + q : OUT_F], in_=outrr)

    _legalize_single_wait(nc)
    return nc


def make_in_maps(x, diagonals, bias):
    x = np.asarray(x, dtype=np.float32)
    diagonals = np.asarray(diagonals, dtype=np.float32)
    bias = np.asarray(bias, dtype=np.float32)

    tbls = _build_tables(diagonals.astype(np.float64))
    bias_row = np.ascontiguousarray(bias.astype(np.float16).reshape(1, OUT_F))
    x16 = x.reshape(ROWS, IN_F).astype(ml_dtypes.bfloat16)

    in_maps = []
    for c in range(N_CORES):
        xc = x16[c * M_PER_CORE : (c + 1) * M_PER_CORE]  # [1024, 4096]
        x4 = xc.reshape(N_MC, MT, N_KC, 128)  # [mc, m, kc, p]
        x4 = np.ascontiguousarray(x4.transpose(0, 3, 2, 1))  # [mc, p, kc, m]
        in_maps.append({"xt": x4, "tbl": tbls, "bias_row": bias_row})
    return in_maps


def kernel(x, diagonals, bias):
    global _COMPILED
    if _COMPILED is None:
        _COMPILED = build_nc()
    nc = _COMPILED

    in_maps = make_in_maps(x, diagonals, bias)
    res = run_bass_kernel_spmd(nc, in_maps, core_ids=list(range(N_CORES)))
    y = np.concatenate(
        [np.asarray(res.results[c]["y"]) for c in range(N_CORES)], axis=0
    )
    return y.astype(np.float32).reshape(B, S, OUT_F)
